# revision 4
# baseline (speedup 1.0000x reference)
"""GraphSAGE-style encoder kernel for Trainium2 (Bass/Tile), 8-core SPMD.

Reference computation (see reference.py):
    neigh_feats = features[neigh_idx].mean(axis=1)    # [B, F]
    self_feats  = features[nodes]                     # [B, F]
    combined    = concat([self_feats, neigh_feats], 1)  # [B, 2F]
    out = relu(weight @ combined.T)                   # [E, B]

Sharding: data-parallel over the batch-of-nodes dim. Each of the 8 cores
gets B/8 = 6250 nodes (padded to 6272 = 49*128), the full feature table and
the full weight. No collectives.

Per-core kernel, per 128-node tile:
  1. one indirect DMA gathers 128*(1 self + 10 neigh) = 1408 rows of 1KB
     into an SBUF tile G[128, 11, 256]
  2. DVE reduces G[:, 1:11, :] over the sample axis -> nsum[128, 256]
  3. PE transposes (via identity matmul) self/nsum chunks into PSUM,
     giving Ct[128(k), 512... 4 chunks of [128, 128]] = combined.T tiles
  4. 8 accumulating fp32 matmuls: out[m-chunk] += wT[k,m].T @ Ct[k]
     (weight pre-transposed/pre-scaled on host; neigh half carries the /10)
  5. ReLU on ScalarE PSUM->SBUF, DMA out

The weight is fed pre-arranged as wt[p, k*2+m, j] = wT_scaled[k*128+p, m*128+j],
with wT_scaled = weight.T and rows 256: scaled by 1/10 (folds the neighbor
mean into the matmul).
"""

import numpy as np

N_NODES = 100000
FEAT = 256
EMB = 256
B = 50000
S = 10
N_CORES = 8
B_LOC = B // N_CORES          # 6250
TILE = 128
NT = -(-B_LOC // TILE)        # 49 tiles
B_PAD = NT * TILE             # 6272

_NC_CACHE = {}


def build_nc():
    """Build the per-core Bass program (identical on all 8 cores)."""
    import concourse.bass as bass
    import concourse.tile as tile
    from concourse import bacc, mybir
    from concourse.masks import make_identity

    dt = mybir.dt

    nc = bacc.Bacc(None, target_bir_lowering=False)
    f = nc.dram_tensor("features", [N_NODES, FEAT], dt.float32, kind="ExternalInput")
    wt = nc.dram_tensor("wt", [128, 8, 128], dt.float32, kind="ExternalInput")
    idx = nc.dram_tensor("idx", [128, NT, S + 1], dt.int32, kind="ExternalInput")
    o = nc.dram_tensor("out", [EMB, B_PAD], dt.float32, kind="ExternalOutput")

    with tile.TileContext(nc) as tc:
        with (
            tc.tile_pool(name="const", bufs=1) as cpool,
            tc.tile_pool(name="g", bufs=4) as gpool,
            tc.tile_pool(name="nsum", bufs=3) as npool,
            tc.tile_pool(name="rhs", bufs=3) as rpool,
            tc.tile_pool(name="osb", bufs=3) as opool,
            tc.tile_pool(name="tps", bufs=2, space="PSUM") as tps_pool,
            tc.tile_pool(name="ops", bufs=2, space="PSUM") as ops_pool,
        ):
            ident = cpool.tile([128, 128], dt.float32)
            make_identity(nc, ident[:])

            wt_sb = cpool.tile([128, 8, 128], dt.float32)
            nc.sync.dma_start(out=wt_sb[:], in_=wt[:])

            idx_sb = cpool.tile([128, NT, S + 1], dt.int32)
            nc.sync.dma_start(out=idx_sb[:], in_=idx[:])

            # out viewed as [p, m, n]: o[m*128+p, n]
            o_r = o[:].rearrange("(m p) n -> p m n", m=2)

            for t in range(NT):
                g = gpool.tile([128, S + 1, FEAT], dt.float32)
                # HW DGE supports exactly one index per partition per
                # instruction -> 11 canonical gathers per node tile.
                for s in range(S + 1):
                    nc.gpsimd.indirect_dma_start(
                        out=g[:, s, :],
                        out_offset=None,
                        in_=f[:],
                        in_offset=bass.IndirectOffsetOnAxis(
                            ap=idx_sb[:, t, s : s + 1], axis=0
                        ),
                    )

                nsum = npool.tile([128, FEAT], dt.float32)
                nc.vector.tensor_reduce(
                    out=nsum[:],
                    in_=g[:, 1:, :].rearrange("p s f -> p f s"),
                    axis=mybir.AxisListType.X,
                    op=mybir.AluOpType.add,
                )

                # transposed combined chunks: [k-chunk on partitions, nodes]
                t_ps = tps_pool.tile([128, 512], dt.float32, space="PSUM")
                nc.tensor.transpose(
                    out=t_ps[:, 0:128], in_=g[:, 0, 0:128], identity=ident[:]
                )
                nc.tensor.transpose(
                    out=t_ps[:, 128:256], in_=g[:, 0, 128:256], identity=ident[:]
                )
                nc.tensor.transpose(
                    out=t_ps[:, 256:384], in_=nsum[:, 0:128], identity=ident[:]
                )
                nc.tensor.transpose(
                    out=t_ps[:, 384:512], in_=nsum[:, 128:256], identity=ident[:]
                )
                rhs = rpool.tile([128, 512], dt.float32)
                nc.vector.tensor_copy(out=rhs[:], in_=t_ps[:])

                o_ps = ops_pool.tile([128, 256], dt.float32, space="PSUM")
                for m in range(2):
                    for k in range(4):
                        nc.tensor.matmul(
                            out=o_ps[:, m * 128 : (m + 1) * 128],
                            lhsT=wt_sb[:, k * 2 + m, :],
                            rhs=rhs[:, k * 128 : (k + 1) * 128],
                            start=(k == 0),
                            stop=(k == 3),
                        )

                osb = opool.tile([128, 256], dt.float32)
                nc.scalar.activation(
                    out=osb[:],
                    in_=o_ps[:],
                    func=mybir.ActivationFunctionType.Relu,
                )
                nc.sync.dma_start(
                    out=o_r[:, :, t * 128 : (t + 1) * 128],
                    in_=osb[:].rearrange("p (m n) -> p m n", m=2),
                )
    nc.compile()
    return nc


def get_nc():
    if "nc" not in _NC_CACHE:
        _NC_CACHE["nc"] = build_nc()
    return _NC_CACHE["nc"]


def prep_in_maps(features, weight, nodes, neigh_idx):
    """Shard/massage the full inputs into the 8 per-core in_maps."""
    features = np.ascontiguousarray(np.asarray(features, dtype=np.float32))
    weight = np.asarray(weight, dtype=np.float32)
    nodes = np.asarray(nodes)
    neigh_idx = np.asarray(neigh_idx)

    # weight.T with the neighbor half pre-scaled by 1/S (the mean),
    # rearranged to [p, k*2+m, j] = wT[k*128+p, m*128+j]
    wTs = weight.T.astype(np.float32).copy()          # [2F, E] = [512, 256]
    wTs[FEAT:, :] *= np.float32(1.0 / S)
    wt_np = np.ascontiguousarray(
        wTs.reshape(4, 128, 2, 128).transpose(1, 0, 2, 3).reshape(128, 8, 128)
    )

    idx_full = np.concatenate(
        [nodes[:, None], neigh_idx], axis=1
    ).astype(np.int32)                                # [B, 11]

    in_maps = []
    for c in range(N_CORES):
        idx_c = idx_full[c * B_LOC : (c + 1) * B_LOC]
        idx_c = np.pad(idx_c, ((0, B_PAD - B_LOC), (0, 0)))
        idx_c = np.ascontiguousarray(
            idx_c.reshape(NT, 128, S + 1).transpose(1, 0, 2)
        )                                             # [128, NT, 11]
        in_maps.append({"features": features, "wt": wt_np, "idx": idx_c})
    return in_maps


def postprocess(results):
    outs = [np.asarray(r["out"])[:, :B_LOC] for r in results]
    return np.concatenate(outs, axis=1)


def kernel(features, weight, nodes, neigh_idx):
    from concourse.bass_utils import run_bass_kernel_spmd

    nc = get_nc()
    in_maps = prep_in_maps(features, weight, nodes, neigh_idx)
    res = run_bass_kernel_spmd(nc, in_maps, core_ids=list(range(N_CORES)))
    return postprocess(res.results)


# revision 5
# speedup vs baseline: 1.0008x; 1.0008x over previous
"""GraphSAGE-style encoder kernel for Trainium2 (Bass/Tile), 8-core SPMD.

Reference computation (see reference.py):
    neigh_feats = features[neigh_idx].mean(axis=1)    # [B, F]
    self_feats  = features[nodes]                     # [B, F]
    combined    = concat([self_feats, neigh_feats], 1)  # [B, 2F]
    out = relu(weight @ combined.T)                   # [E, B]

Sharding: data-parallel over the batch-of-nodes dim. Each of the 8 cores
gets B/8 = 6250 nodes (padded to 6272 = 49*128), the full feature table and
the full weight. No collectives.

Per-core kernel, per 128-node tile:
  1. one indirect DMA gathers 128*(1 self + 10 neigh) = 1408 rows of 1KB
     into an SBUF tile G[128, 11, 256]
  2. DVE reduces G[:, 1:11, :] over the sample axis -> nsum[128, 256]
  3. PE transposes (via identity matmul) self/nsum chunks into PSUM,
     giving Ct[128(k), 512... 4 chunks of [128, 128]] = combined.T tiles
  4. 8 accumulating fp32 matmuls: out[m-chunk] += wT[k,m].T @ Ct[k]
     (weight pre-transposed/pre-scaled on host; neigh half carries the /10)
  5. ReLU on ScalarE PSUM->SBUF, DMA out

The weight is fed pre-arranged as wt[p, k*2+m, j] = wT_scaled[k*128+p, m*128+j],
with wT_scaled = weight.T and rows 256: scaled by 1/10 (folds the neighbor
mean into the matmul).
"""

import numpy as np

N_NODES = 100000
FEAT = 256
EMB = 256
B = 50000
S = 10
N_CORES = 8
B_LOC = B // N_CORES          # 6250
TILE = 128
NT = -(-B_LOC // TILE)        # 49 tiles
B_PAD = NT * TILE             # 6272

_NC_CACHE = {}


def build_nc():
    """Build the per-core Bass program (identical on all 8 cores)."""
    import concourse.bass as bass
    import concourse.tile as tile
    from concourse import bacc, mybir
    from concourse.masks import make_identity

    dt = mybir.dt

    nc = bacc.Bacc(None, target_bir_lowering=False)
    f = nc.dram_tensor("features", [N_NODES, FEAT], dt.float32, kind="ExternalInput")
    wt = nc.dram_tensor("wt", [128, 8, 128], dt.float32, kind="ExternalInput")
    idx = nc.dram_tensor("idx", [128, NT, S + 1], dt.int32, kind="ExternalInput")
    o = nc.dram_tensor("out", [EMB, B_PAD], dt.float32, kind="ExternalOutput")

    with tile.TileContext(nc) as tc:
        with (
            tc.tile_pool(name="const", bufs=1) as cpool,
            tc.tile_pool(name="g", bufs=6) as gpool,
            tc.tile_pool(name="nsum", bufs=3) as npool,
            tc.tile_pool(name="rhs", bufs=3) as rpool,
            tc.tile_pool(name="osb", bufs=3) as opool,
            tc.tile_pool(name="tps", bufs=2, space="PSUM") as tps_pool,
            tc.tile_pool(name="ops", bufs=2, space="PSUM") as ops_pool,
        ):
            ident = cpool.tile([128, 128], dt.float32)
            make_identity(nc, ident[:])

            wt_sb = cpool.tile([128, 8, 128], dt.float32)
            nc.sync.dma_start(out=wt_sb[:], in_=wt[:])

            idx_sb = cpool.tile([128, NT, S + 1], dt.int32)
            nc.sync.dma_start(out=idx_sb[:], in_=idx[:])

            # out viewed as [p, m, n]: o[m*128+p, n]
            o_r = o[:].rearrange("(m p) n -> p m n", m=2)

            for t in range(NT):
                g = gpool.tile([128, S + 1, FEAT], dt.float32)
                # HW DGE supports exactly one index per partition per
                # instruction -> 11 canonical gathers per node tile.
                for s in range(S + 1):
                    nc.gpsimd.indirect_dma_start(
                        out=g[:, s, :],
                        out_offset=None,
                        in_=f[:],
                        in_offset=bass.IndirectOffsetOnAxis(
                            ap=idx_sb[:, t, s : s + 1], axis=0
                        ),
                    )

                nsum = npool.tile([128, FEAT], dt.float32)
                nc.vector.tensor_reduce(
                    out=nsum[:],
                    in_=g[:, 1:, :].rearrange("p s f -> p f s"),
                    axis=mybir.AxisListType.X,
                    op=mybir.AluOpType.add,
                )

                # transposed combined chunks: [k-chunk on partitions, nodes]
                t_ps = tps_pool.tile([128, 512], dt.float32, space="PSUM")
                nc.tensor.transpose(
                    out=t_ps[:, 0:128], in_=g[:, 0, 0:128], identity=ident[:]
                )
                nc.tensor.transpose(
                    out=t_ps[:, 128:256], in_=g[:, 0, 128:256], identity=ident[:]
                )
                nc.tensor.transpose(
                    out=t_ps[:, 256:384], in_=nsum[:, 0:128], identity=ident[:]
                )
                nc.tensor.transpose(
                    out=t_ps[:, 384:512], in_=nsum[:, 128:256], identity=ident[:]
                )
                rhs = rpool.tile([128, 512], dt.float32)
                nc.vector.tensor_copy(out=rhs[:], in_=t_ps[:])

                o_ps = ops_pool.tile([128, 256], dt.float32, space="PSUM")
                for m in range(2):
                    for k in range(4):
                        nc.tensor.matmul(
                            out=o_ps[:, m * 128 : (m + 1) * 128],
                            lhsT=wt_sb[:, k * 2 + m, :],
                            rhs=rhs[:, k * 128 : (k + 1) * 128],
                            start=(k == 0),
                            stop=(k == 3),
                        )

                osb = opool.tile([128, 256], dt.float32)
                nc.scalar.activation(
                    out=osb[:],
                    in_=o_ps[:],
                    func=mybir.ActivationFunctionType.Relu,
                )
                nc.sync.dma_start(
                    out=o_r[:, :, t * 128 : (t + 1) * 128],
                    in_=osb[:].rearrange("p (m n) -> p m n", m=2),
                )
    nc.compile()
    return nc


def get_nc():
    if "nc" not in _NC_CACHE:
        _NC_CACHE["nc"] = build_nc()
    return _NC_CACHE["nc"]


def prep_in_maps(features, weight, nodes, neigh_idx):
    """Shard/massage the full inputs into the 8 per-core in_maps."""
    features = np.ascontiguousarray(np.asarray(features, dtype=np.float32))
    weight = np.asarray(weight, dtype=np.float32)
    nodes = np.asarray(nodes)
    neigh_idx = np.asarray(neigh_idx)

    # weight.T with the neighbor half pre-scaled by 1/S (the mean),
    # rearranged to [p, k*2+m, j] = wT[k*128+p, m*128+j]
    wTs = weight.T.astype(np.float32).copy()          # [2F, E] = [512, 256]
    wTs[FEAT:, :] *= np.float32(1.0 / S)
    wt_np = np.ascontiguousarray(
        wTs.reshape(4, 128, 2, 128).transpose(1, 0, 2, 3).reshape(128, 8, 128)
    )

    idx_full = np.concatenate(
        [nodes[:, None], neigh_idx], axis=1
    ).astype(np.int32)                                # [B, 11]

    in_maps = []
    for c in range(N_CORES):
        idx_c = idx_full[c * B_LOC : (c + 1) * B_LOC]
        idx_c = np.pad(idx_c, ((0, B_PAD - B_LOC), (0, 0)))
        idx_c = np.ascontiguousarray(
            idx_c.reshape(NT, 128, S + 1).transpose(1, 0, 2)
        )                                             # [128, NT, 11]
        in_maps.append({"features": features, "wt": wt_np, "idx": idx_c})
    return in_maps


def postprocess(results):
    outs = [np.asarray(r["out"])[:, :B_LOC] for r in results]
    return np.concatenate(outs, axis=1)


def kernel(features, weight, nodes, neigh_idx):
    from concourse.bass_utils import run_bass_kernel_spmd

    nc = get_nc()
    in_maps = prep_in_maps(features, weight, nodes, neigh_idx)
    res = run_bass_kernel_spmd(nc, in_maps, core_ids=list(range(N_CORES)))
    return postprocess(res.results)


# revision 7
# speedup vs baseline: 1.0065x; 1.0057x over previous
"""GraphSAGE-style encoder kernel for Trainium2 (Bass/Tile), 8-core SPMD.

Reference computation (see reference.py):
    neigh_feats = features[neigh_idx].mean(axis=1)    # [B, F]
    self_feats  = features[nodes]                     # [B, F]
    combined    = concat([self_feats, neigh_feats], 1)  # [B, 2F]
    out = relu(weight @ combined.T)                   # [E, B]

Sharding: data-parallel over the batch-of-nodes dim. Each of the 8 cores
gets B/8 = 6250 nodes (padded to 6272 = 49*128), the full feature table and
the full weight. No collectives.

Per-core kernel, per 128-node tile:
  1. one indirect DMA gathers 128*(1 self + 10 neigh) = 1408 rows of 1KB
     into an SBUF tile G[128, 11, 256]
  2. DVE reduces G[:, 1:11, :] over the sample axis -> nsum[128, 256]
  3. PE transposes (via identity matmul) self/nsum chunks into PSUM,
     giving Ct[128(k), 512... 4 chunks of [128, 128]] = combined.T tiles
  4. 8 accumulating fp32 matmuls: out[m-chunk] += wT[k,m].T @ Ct[k]
     (weight pre-transposed/pre-scaled on host; neigh half carries the /10)
  5. ReLU on ScalarE PSUM->SBUF, DMA out

The weight is fed pre-arranged as wt[p, k*2+m, j] = wT_scaled[k*128+p, m*128+j],
with wT_scaled = weight.T and rows 256: scaled by 1/10 (folds the neighbor
mean into the matmul).
"""

import numpy as np

N_NODES = 100000
FEAT = 256
EMB = 256
B = 50000
S = 10
N_CORES = 8
B_LOC = B // N_CORES          # 6250
TILE = 128
NT = -(-B_LOC // TILE)        # 49 tiles
B_PAD = NT * TILE             # 6272

_NC_CACHE = {}


def build_nc():
    """Build the per-core Bass program (identical on all 8 cores)."""
    import concourse.bass as bass
    import concourse.tile as tile
    from concourse import bacc, mybir
    from concourse.masks import make_identity

    dt = mybir.dt

    nc = bacc.Bacc(None, target_bir_lowering=False)
    f = nc.dram_tensor("features", [N_NODES, FEAT], dt.float32, kind="ExternalInput")
    wt = nc.dram_tensor("wt", [128, 8, 128], dt.float32, kind="ExternalInput")
    idx = nc.dram_tensor("idx", [128, NT, S + 1], dt.int32, kind="ExternalInput")
    o = nc.dram_tensor("out", [EMB, B_PAD], dt.float32, kind="ExternalOutput")

    with tile.TileContext(nc) as tc:
        with (
            tc.tile_pool(name="const", bufs=1) as cpool,
            tc.tile_pool(name="g", bufs=6) as gpool,
            tc.tile_pool(name="nsum", bufs=3) as npool,
            tc.tile_pool(name="rhs", bufs=3) as rpool,
            tc.tile_pool(name="osb", bufs=3) as opool,
            tc.tile_pool(name="tps", bufs=2, space="PSUM") as tps_pool,
            tc.tile_pool(name="ops", bufs=2, space="PSUM") as ops_pool,
        ):
            ident = cpool.tile([128, 128], dt.float32)
            make_identity(nc, ident[:])

            wt_sb = cpool.tile([128, 8, 128], dt.float32)
            nc.sync.dma_start(out=wt_sb[:], in_=wt[:])

            # split idx load so tile 0's gathers start ~2.5us sooner
            idx_sb = cpool.tile([128, NT, S + 1], dt.int32)
            nc.sync.dma_start(out=idx_sb[:, 0:1, :], in_=idx[:, 0:1, :])
            nc.sync.dma_start(out=idx_sb[:, 1:, :], in_=idx[:, 1:, :])

            # out viewed as [p, m, n]: o[m*128+p, n]
            o_r = o[:].rearrange("(m p) n -> p m n", m=2)

            for t in range(NT):
                g = gpool.tile([128, S + 1, FEAT], dt.float32)
                # HW DGE supports exactly one index per partition per
                # instruction -> 11 canonical gathers per node tile.
                # Neighbors first, self (s=0) last: the neighbor sum is
                # accumulated incrementally as each gather lands, so only
                # one ~0.3us add trails the final neighbor gather.
                for s in range(1, S + 1):
                    nc.gpsimd.indirect_dma_start(
                        out=g[:, s, :],
                        out_offset=None,
                        in_=f[:],
                        in_offset=bass.IndirectOffsetOnAxis(
                            ap=idx_sb[:, t, s : s + 1], axis=0
                        ),
                    )
                nc.gpsimd.indirect_dma_start(
                    out=g[:, 0, :],
                    out_offset=None,
                    in_=f[:],
                    in_offset=bass.IndirectOffsetOnAxis(
                        ap=idx_sb[:, t, 0:1], axis=0
                    ),
                )

                nsum = npool.tile([128, FEAT], dt.float32)
                nc.vector.tensor_tensor(
                    out=nsum[:], in0=g[:, 1, :], in1=g[:, 2, :],
                    op=mybir.AluOpType.add,
                )
                for s in range(3, S + 1):
                    nc.vector.tensor_tensor(
                        out=nsum[:], in0=nsum[:], in1=g[:, s, :],
                        op=mybir.AluOpType.add,
                    )

                # transposed combined chunks: [k-chunk on partitions, nodes]
                t_ps = tps_pool.tile([128, 512], dt.float32, space="PSUM")
                nc.tensor.transpose(
                    out=t_ps[:, 0:128], in_=g[:, 0, 0:128], identity=ident[:]
                )
                nc.tensor.transpose(
                    out=t_ps[:, 128:256], in_=g[:, 0, 128:256], identity=ident[:]
                )
                nc.tensor.transpose(
                    out=t_ps[:, 256:384], in_=nsum[:, 0:128], identity=ident[:]
                )
                nc.tensor.transpose(
                    out=t_ps[:, 384:512], in_=nsum[:, 128:256], identity=ident[:]
                )
                rhs = rpool.tile([128, 512], dt.float32)
                nc.vector.tensor_copy(out=rhs[:], in_=t_ps[:])

                o_ps = ops_pool.tile([128, 256], dt.float32, space="PSUM")
                for m in range(2):
                    for k in range(4):
                        nc.tensor.matmul(
                            out=o_ps[:, m * 128 : (m + 1) * 128],
                            lhsT=wt_sb[:, k * 2 + m, :],
                            rhs=rhs[:, k * 128 : (k + 1) * 128],
                            start=(k == 0),
                            stop=(k == 3),
                        )

                osb = opool.tile([128, 256], dt.float32)
                nc.scalar.activation(
                    out=osb[:],
                    in_=o_ps[:],
                    func=mybir.ActivationFunctionType.Relu,
                )
                nc.sync.dma_start(
                    out=o_r[:, :, t * 128 : (t + 1) * 128],
                    in_=osb[:].rearrange("p (m n) -> p m n", m=2),
                )
    nc.compile()
    return nc


def get_nc():
    if "nc" not in _NC_CACHE:
        _NC_CACHE["nc"] = build_nc()
    return _NC_CACHE["nc"]


def prep_in_maps(features, weight, nodes, neigh_idx):
    """Shard/massage the full inputs into the 8 per-core in_maps."""
    features = np.ascontiguousarray(np.asarray(features, dtype=np.float32))
    weight = np.asarray(weight, dtype=np.float32)
    nodes = np.asarray(nodes)
    neigh_idx = np.asarray(neigh_idx)

    # weight.T with the neighbor half pre-scaled by 1/S (the mean),
    # rearranged to [p, k*2+m, j] = wT[k*128+p, m*128+j]
    wTs = weight.T.astype(np.float32).copy()          # [2F, E] = [512, 256]
    wTs[FEAT:, :] *= np.float32(1.0 / S)
    wt_np = np.ascontiguousarray(
        wTs.reshape(4, 128, 2, 128).transpose(1, 0, 2, 3).reshape(128, 8, 128)
    )

    idx_full = np.concatenate(
        [nodes[:, None], neigh_idx], axis=1
    ).astype(np.int32)                                # [B, 11]

    in_maps = []
    for c in range(N_CORES):
        idx_c = idx_full[c * B_LOC : (c + 1) * B_LOC]
        idx_c = np.pad(idx_c, ((0, B_PAD - B_LOC), (0, 0)))
        idx_c = np.ascontiguousarray(
            idx_c.reshape(NT, 128, S + 1).transpose(1, 0, 2)
        )                                             # [128, NT, 11]
        in_maps.append({"features": features, "wt": wt_np, "idx": idx_c})
    return in_maps


def postprocess(results):
    outs = [np.asarray(r["out"])[:, :B_LOC] for r in results]
    return np.concatenate(outs, axis=1)


def kernel(features, weight, nodes, neigh_idx):
    from concourse.bass_utils import run_bass_kernel_spmd

    nc = get_nc()
    in_maps = prep_in_maps(features, weight, nodes, neigh_idx)
    res = run_bass_kernel_spmd(nc, in_maps, core_ids=list(range(N_CORES)))
    return postprocess(res.results)
